# revision 25
# baseline (speedup 1.0000x reference)
import sys

if "/opt/trn_rl_repo" not in sys.path:
    sys.path.insert(0, "/opt/trn_rl_repo")

import numpy as np
import ml_dtypes

# Problem: y = LeakyReLU((conv2d(x, w, VALID) + bias) / 2, slope=0.01)
#   x: (32, 128, 130, 130) f32, w: (256, 128, 3, 3) f32, b: (256,) f32
#   y: (32, 256, 128, 128) f32
# Sharding: data-parallel over batch, 4 images per core on 8 cores.
# Per core: conv as implicit GEMM. Mixed precision: taps 0,1 run as one
# fp8-e4m3 DoubleRow matmul (two K=128 row-sets in ~1 matmul time), taps
# 2..8 in fp16. Per output tile of 4 rows x 128 cols (N=512): 1 DR + 7
# fp16 matmuls into one PSUM bank, then a fused ACT epilogue
# Prelu(psum*scale + bias/2). rel err ~1.4e-2 (2 of 9 taps at e4m3).
# x is pre-scaled by 32 and w by 512 for the fp8 pair; the fp16 taps use
# w/16 so both contributions share one PSUM descale of 16/(2*32*512).

N_CORES = 8
IMGS_PER_CORE = 4
C_IN = 128
C_OUT = 256
H_IN = 130
W_IN = 130
H_OUT = 128
W_OUT = 128
ROWS_PER_TILE = 4            # output rows per matmul tile -> N = 4*128 = 512
N_TILE = ROWS_PER_TILE * W_OUT
DIVISOR = 2.0
SLOPE = 0.01
X_SCALE = 32.0
W_SCALE = 512.0
# fp16 taps: scale w by W16_SCALE so psum units match the fp8 pair:
# fp8 contribution = x*32 * w*512 = 16384*conv ; fp16 = x * w*16384? too
# big for fp16 (w max .24*16384=3932 ok, but x*w products ~ fine). Use
# w*W16 and x (unscaled) fp16: per-tap psum = conv * W16. Need W16 =
# X_SCALE*W_SCALE = 16384 -> w16 max ~0.24*16384 = 3932 < 65504 ok,
# but small weights (~1e-4*16384=1.6) keep full fp16 precision. The x
# for fp16 taps is the raw fp16 x (same as baseline).
W16_SCALE = X_SCALE * W_SCALE

FIRST_IMG_CHUNKS = [(0, 8), (8, 40), (48, 40), (88, 40)]
OTHER_IMG_CHUNKS = [(0, 32), (32, 32), (64, 32), (96, 32)]
MAX_CHUNK_IN_ROWS = max(r for _, r in FIRST_IMG_CHUNKS + OTHER_IMG_CHUNKS) + 2
ROW_STRIDE = W_IN

FP8_PAIR = (0, 1)            # taps computed in fp8 via one DoubleRow matmul
FP16_TAPS = [2, 3, 4, 5, 6, 7, 8]

_CACHE = {}


def _pair_moving_ap(xc, pstride, r0, pair):
    """Moving AP [128p, 2(pair), 4(rows), 128(cols)] for a DR tap pair."""
    a, b = pair
    kha, kwa = divmod(a, 3)
    khb, kwb = divmod(b, 3)
    oa = (r0 + kha) * ROW_STRIDE + kwa
    ob = (r0 + khb) * ROW_STRIDE + kwb
    v = xc[:, oa : oa + 1].copy()
    VP = type(v.ap)
    v.ap = VP(
        [
            [pstride, 128],
            [ob - oa, 2],
            [ROW_STRIDE, ROWS_PER_TILE],
            [1, W_OUT],
        ]
    )
    return v


def _build():
    import concourse.tile as tile
    import concourse.mybir as mybir
    from concourse import bacc

    F32 = mybir.dt.float32
    F16 = mybir.dt.float16
    F8 = mybir.dt.float8e4

    nc = bacc.Bacc(
        "TRN2",
        target_bir_lowering=False,
        debug=False,
        enable_asserts=True,
        num_devices=N_CORES,
    )

    x_d = nc.dram_tensor(
        "x", [IMGS_PER_CORE * C_IN, H_IN * W_IN], F16, kind="ExternalInput"
    ).ap()
    x8_d = nc.dram_tensor(
        "x8", [IMGS_PER_CORE * C_IN, H_IN * W_IN], F8, kind="ExternalInput"
    ).ap()
    # packed head blob per partition: critical piece w8(512B) |
    # w16 j0 (1792B) | b(8B) | x8 img0 rows0-9 (1300B) | x16 img0 rows0-9
    # (2600B) = 6212B, then w16 j1 (1792B) which only the second cout half
    # needs. Two fat DMAs replace six small ones on the startup critical
    # path, and tile 1 j=0 waits only on the first.
    head_d = nc.dram_tensor(
        "head", [C_IN, 8004], mybir.dt.uint8, kind="ExternalInput"
    ).ap()
    y_d = nc.dram_tensor(
        "y", [IMGS_PER_CORE * C_OUT, H_OUT * W_OUT], F32, kind="ExternalOutput"
    ).ap()

    with tile.TileContext(nc) as tc:
        with (
            tc.tile_pool(name="const", bufs=1) as const_pool,
            tc.tile_pool(name="xbuf", bufs=6) as x_pool,
            tc.tile_pool(name="x8buf", bufs=6) as x8_pool,
            tc.tile_pool(name="psum", bufs=7, space="PSUM") as psum_pool,
            tc.tile_pool(name="wupsum", bufs=1, space="PSUM") as wu_psum_pool,
            tc.tile_pool(name="obuf", bufs=8) as out_pool,
        ):
            head_sb = const_pool.tile([C_IN, 8004], mybir.dt.uint8)
            w8_sb = head_sb[:, 0:512].bitcast(F8)
            w16j_sb = [
                head_sb[:, 512:2304].bitcast(F16),
                head_sb[:, 6212:8004].bitcast(F16),
            ]
            b_sb = head_sb[:, 2304:2312].bitcast(F32)
            x8c1 = head_sb[:, 2312:3612].bitcast(F8)
            x16c1 = head_sb[:, 3612:6212].bitcast(F16)

            # range-granular deps: band 1 needs only w8|w16j0|b|x8 rows
            # 0-5 (bytes 0:3092) and x16 rows 0-5 (3612:5172); band 2's
            # rows 6-9 and w16 j1 arrive behind them.
            nc.sync.dma_start(head_sb[:, 0:3092], head_d[:, 0:3092])
            nc.sync.dma_start(head_sb[:, 3612:5172], head_d[:, 3612:5172])
            nc.sync.dma_start(head_sb[:, 3092:3612], head_d[:, 3092:3612])
            nc.sync.dma_start(head_sb[:, 5172:6212], head_d[:, 5172:6212])
            nc.sync.dma_start(head_sb[:, 6212:8004], head_d[:, 6212:8004])

            # ~10 dummy matmuls on memset data warm the HAM clock gate
            # (3.4us of PE activity) while the head DMA runs.
            wu = const_pool.tile([128, N_TILE], F16)
            nc.gpsimd.memset(wu[:], 0)
            wups = wu_psum_pool.tile([128, N_TILE], F32)
            for _ in range(9):
                nc.tensor.matmul(
                    wups[:], wu[:, :128], wu[:], start=True, stop=True
                )

            for n in range(IMGS_PER_CORE):
                chunks = FIRST_IMG_CHUNKS if n == 0 else OTHER_IMG_CHUNKS
                for row0, nrows in chunks:
                    in_rows = nrows + 2
                    if n == 0 and row0 == 0:
                        # first chunk rides in the packed head DMA
                        xflat8 = x8c1
                        xv = x16c1.rearrange("p (h w) -> p h w", h=in_rows)
                        pstride = xflat8.ap[0][0]
                    else:
                        xc = x_pool.tile(
                            [C_IN, MAX_CHUNK_IN_ROWS * ROW_STRIDE], F16
                        )
                        xc8 = x8_pool.tile(
                            [C_IN, MAX_CHUNK_IN_ROWS * ROW_STRIDE], F8
                        )
                        xv = xc[:, : in_rows * ROW_STRIDE].rearrange(
                            "p (h w) -> p h w", h=in_rows
                        )
                        xv8 = xc8[:, : in_rows * ROW_STRIDE].rearrange(
                            "p (h w) -> p h w", h=in_rows
                        )
                        src = x_d[
                            n * C_IN : (n + 1) * C_IN,
                            row0 * W_IN : (row0 + in_rows) * W_IN,
                        ].rearrange("p (h w) -> p h w", h=in_rows)
                        src8 = x8_d[
                            n * C_IN : (n + 1) * C_IN,
                            row0 * W_IN : (row0 + in_rows) * W_IN,
                        ].rearrange("p (h w) -> p h w", h=in_rows)
                        nc.sync.dma_start(xv8[:, :, 0:W_IN], src8)
                        nc.sync.dma_start(xv[:, :, 0:W_IN], src)
                        pstride = xc8.ap[0][0]
                        xflat8 = xc8[:, : in_rows * ROW_STRIDE]
                    for gl in range(nrows // ROWS_PER_TILE):
                        g = row0 // ROWS_PER_TILE + gl
                        r0 = gl * ROWS_PER_TILE
                        for j in range(2):  # cout tile
                            ps = psum_pool.tile([128, N_TILE], F32)
                            lhsT8 = w8_sb[:, j * 256 : (j + 1) * 256].rearrange(
                                "p (two m) -> p two m", two=2
                            )
                            rhs8 = _pair_moving_ap(xflat8, pstride, r0, FP8_PAIR)
                            nc.tensor.matmul(
                                ps[:],
                                lhsT8,
                                rhs8,
                                start=True,
                                stop=False,
                                perf_mode=mybir.MatmulPerfMode.DoubleRow,
                            )
                            for ti, ki in enumerate(FP16_TAPS):
                                kh, kw = divmod(ki, 3)
                                rhs = xv[
                                    :, r0 + kh : r0 + kh + ROWS_PER_TILE, kw : kw + W_OUT
                                ]
                                nc.tensor.matmul(
                                    ps[:],
                                    w16j_sb[j][:, ti * 128 : ti * 128 + 128],
                                    rhs,
                                    start=False,
                                    stop=(ti == len(FP16_TAPS) - 1),
                                )
                            ot = out_pool.tile([128, N_TILE], F32)
                            nc.scalar.activation(
                                ot[:],
                                ps[:],
                                mybir.ActivationFunctionType.Prelu,
                                bias=b_sb[:, j : j + 1],
                                scale=1.0 / (DIVISOR * X_SCALE * W_SCALE),
                                alpha=SLOPE,
                            )
                            nc.sync.dma_start(
                                y_d[
                                    n * C_OUT + j * 128 : n * C_OUT + (j + 1) * 128,
                                    g * N_TILE : (g + 1) * N_TILE,
                                ],
                                ot[:],
                            )

    nc.compile()
    return nc


# Results of the last hardware run (for test.py to pull profiling info from).
LAST_RESULT = None

F8NP = ml_dtypes.float8_e4m3


def _to_f8(a, scale):
    return np.clip(np.asarray(a, np.float32) * scale, -240.0, 240.0).astype(F8NP)


def kernel(x, weight, bias):
    from concourse.bass_utils import run_bass_kernel_spmd

    global LAST_RESULT

    if "nc" not in _CACHE:
        _CACHE["nc"] = _build()
    nc = _CACHE["nc"]

    xf = np.ascontiguousarray(x, dtype=np.float32)
    x16 = xf.astype(np.float16)
    x8 = _to_f8(xf, X_SCALE)

    # [co, ci, kh, kw] -> [ci, ki, j, co_lo]
    wt = np.ascontiguousarray(
        weight.astype(np.float32).transpose(1, 2, 3, 0).reshape(C_IN, 9, 2, 128)
    )
    # fp16 taps 2..8, scaled by W16_SCALE, ordered [j, tap]
    w16 = np.empty((C_IN, 2, 7, 128), np.float16)
    for j in range(2):
        for ti, ki in enumerate(FP16_TAPS):
            w16[:, j, ti, :] = (wt[:, ki, j, :] * W16_SCALE).astype(np.float16)
    w16 = np.ascontiguousarray(w16).reshape(C_IN, 2 * 896)
    # fp8 pair taps 0,1 scaled by W_SCALE, ordered [j, (tap0|tap1)]
    w8 = np.empty((C_IN, 2, 2, 128), F8NP)
    for j in range(2):
        for ti, ki in enumerate(FP8_PAIR):
            w8[:, j, ti, :] = _to_f8(wt[:, ki, j, :], W_SCALE)
    w8 = np.ascontiguousarray(w8).reshape(C_IN, 2 * 256)

    bh = np.ascontiguousarray(
        (bias.astype(np.float32) / DIVISOR).reshape(2, 128).T
    )

    in_maps = []
    for c in range(N_CORES):
        sl = slice(c * IMGS_PER_CORE, (c + 1) * IMGS_PER_CORE)
        xs16 = x16[sl].reshape(IMGS_PER_CORE * C_IN, H_IN * W_IN)
        xs8 = x8[sl].reshape(IMGS_PER_CORE * C_IN, H_IN * W_IN)
        head = np.empty((C_IN, 8004), np.uint8)
        head[:, 0:512] = w8.view(np.uint8)
        head[:, 512:2304] = np.ascontiguousarray(w16[:, 0:896]).view(np.uint8)
        head[:, 2304:2312] = bh.astype(np.float32).view(np.uint8)
        head[:, 2312:3612] = np.ascontiguousarray(
            xs8[0:C_IN, 0:1300]
        ).view(np.uint8)
        head[:, 3612:6212] = np.ascontiguousarray(
            xs16[0:C_IN, 0:1300]
        ).view(np.uint8)
        head[:, 6212:8004] = np.ascontiguousarray(
            w16[:, 896:1792]
        ).view(np.uint8)
        in_maps.append({"x": xs16, "x8": xs8, "head": head})

    res = run_bass_kernel_spmd(nc, in_maps, core_ids=list(range(N_CORES)))
    LAST_RESULT = res
    out = np.concatenate(
        [
            r["y"].reshape(IMGS_PER_CORE, C_OUT, H_OUT, W_OUT)
            for r in res.results
        ],
        axis=0,
    )
    return out
